# revision 1
# baseline (speedup 1.0000x reference)
"""NonLocalBlock (single-head attention, N=HW=4096, d=128) on 8 trn2 cores.

Sharding: data-parallel over batch (B=8) — one batch element per NeuronCore.
Per core, the whole block runs out of SBUF:

  xf (256, 4096) -> theta_T = wt@xf + bt      (128, N)   [PE + bias on copy]
                    phi     = wp@xf + bp      (128, N)   [PE + bias on copy]
                    g0      = (wg@xf)^T       (N, 128)   [PE, no bias]
  S^T[m, n] = sum_i phi[i,m] * theta_T[i,n]   (keys m on partitions)
  expS = exp(S^T - 40)                         [ACT]
  sums[n] = sum_m expS[m, n]                   [PE ones-matmul / DVE adds]
  yT[o, n] = (sum_m g0[m,o] expS[m,n]) / sums[n]
  out = wW @ (yT + bg) + bW + xf  ==  wW@yT + (wW@bg + bW) + xf

Softmax is computed without a per-row max: scores are ~N(0, 128) with
empirical |S| < ~91, so exp(S - 40) (a global shift — softmax is
shift-invariant) stays comfortably inside fp32 range: max e^51 ~ 1e22,
and the smallest row max is ~25 -> e^-15, far above underflow.

Matmuls use float32r (fp22 mantissa truncation, 1 PE pass) — rel err ~1e-4.
All matmul-feeding tensors are declared float32r end to end so the BIR
verifier sees rounded producers; numpy side is plain float32.
"""

import numpy as np
from contextlib import ExitStack

import concourse.bass as bass
import concourse.mybir as mybir
import concourse.tile as tile
from concourse import bacc

P = 128          # partitions / inter channels
C = 256          # input channels
F32 = mybir.dt.float32
F32R = mybir.dt.float32r
AF = mybir.ActivationFunctionType
BF16 = mybir.dt.bfloat16
CSHIFT = 40.0    # global score shift before exp (softmax-invariant)

B_FULL = 8
H_FULL = 64
W_FULL = 64
N_FULL = H_FULL * W_FULL


def build_nc(N=N_FULL, NQ=1024, pe_sum_chunks=0):
    """Build the single-core Bass module (SPMD: same NEFF on all 8 cores)."""
    assert N % 512 == 0 and NQ % 512 == 0 and N % NQ == 0
    MC = N // P                   # number of 128-row key chunks
    NB = NQ // 512                # 512-wide matmul blocks per quarter
    NQn = N // NQ                 # query quarters
    pe_mcs = set(range(min(pe_sum_chunks, MC)))

    nc = bacc.Bacc("TRN2", target_bir_lowering=False, debug=False)

    x_d = nc.dram_tensor("x", [C, N], F32R, kind="ExternalInput").ap()
    # weights host-packed to partition-major [128, 2*128] so DMAs are
    # trivially contiguous (one descriptor per partition)
    wtT_d = nc.dram_tensor("wtT", [P, 2 * P], F32R, kind="ExternalInput").ap()
    wpT_d = nc.dram_tensor("wpT", [P, 2 * P], F32R, kind="ExternalInput").ap()
    wgT_d = nc.dram_tensor("wgT", [P, 2 * P], F32R, kind="ExternalInput").ap()
    wWT_d = nc.dram_tensor("wWT", [P, C], F32R, kind="ExternalInput").ap()
    bt_d = nc.dram_tensor("bt", [P, 1], F32, kind="ExternalInput").ap()
    bp_d = nc.dram_tensor("bp", [P, 1], F32, kind="ExternalInput").ap()
    bWp_d = nc.dram_tensor("bWp", [P, 2], F32, kind="ExternalInput").ap()
    out_d = nc.dram_tensor("out", [C, N], F32, kind="ExternalOutput").ap()

    x_v = x_d.rearrange("(k p) n -> k p n", p=P)
    out_v = out_d.rearrange("(k p) n -> k p n", p=P)

    with tile.TileContext(nc) as tc, ExitStack() as ctx:
        const = ctx.enter_context(tc.tile_pool(name="const", bufs=1))
        big = ctx.enter_context(tc.tile_pool(name="big", bufs=1))
        work = ctx.enter_context(tc.tile_pool(name="work", bufs=3))
        ps_bufs = 2 if pe_mcs else 3
        ps = ctx.enter_context(
            tc.tile_pool(name="ps", bufs=ps_bufs, space="PSUM"))
        psy = ctx.enter_context(tc.tile_pool(name="psy", bufs=1, space="PSUM"))

        # ---- constant + input loads ----
        wtT_sb = const.tile([P, 2, P], F32R, name="wtT_sb")
        wpT_sb = const.tile([P, 2, P], F32R, name="wpT_sb")
        wgT_sb = const.tile([P, 2, P], F32R, name="wgT_sb")
        wWT_sb = const.tile([P, C], F32R, name="wWT_sb")
        bt_sb = const.tile([P, 1], F32, name="bt_sb")
        bp_sb = const.tile([P, 1], F32, name="bp_sb")
        bWp_sb = const.tile([P, 2], F32, name="bWp_sb")
        ones_sb = const.tile([P, P], BF16, name="ones_sb")
        cshift_sb = const.tile([P, 1], F32, name="cshift_sb")
        nc.vector.memset(cshift_sb[:], -CSHIFT)

        nc.sync.dma_start(wtT_sb[:], wtT_d.rearrange("p (k i) -> p k i", k=2))
        nc.sync.dma_start(wpT_sb[:], wpT_d.rearrange("p (k i) -> p k i", k=2))
        nc.sync.dma_start(wgT_sb[:], wgT_d.rearrange("p (k i) -> p k i", k=2))
        nc.sync.dma_start(wWT_sb[:], wWT_d)
        nc.sync.dma_start(bt_sb[:], bt_d)
        nc.sync.dma_start(bp_sb[:], bp_d)
        nc.sync.dma_start(bWp_sb[:], bWp_d)
        nc.vector.memset(ones_sb[:], 1.0)

        x_sb = big.tile([P, 2, N], F32R, name="x_sb")
        # chunk the x load so compute can start while later chunks stream in
        for k in range(2):
            for blk in range(N // 512):
                nc.sync.dma_start(
                    x_sb[:, k, blk * 512:(blk + 1) * 512],
                    x_v[k, :, blk * 512:(blk + 1) * 512],
                )

        th_sb = big.tile([P, N], F32R, name="th_sb")   # theta^T (i, n)
        ph_sb = big.tile([P, N], F32R, name="ph_sb")   # phi (i, m)
        g_sb = big.tile([P, MC, P], BF16, name="g_sb")  # g0 (m_in, m_chunk, o)

        # ---- theta_T / phi: wt@x + bt, wp@x + bp ----
        for blk in range(N // 512):
            sl = slice(blk * 512, (blk + 1) * 512)
            th_ps = ps.tile([P, 512], F32, tag="s", name="th_ps")
            nc.tensor.matmul(th_ps[:], wtT_sb[:, 0], x_sb[:, 0, sl],
                             start=True, stop=False)
            nc.tensor.matmul(th_ps[:], wtT_sb[:, 1], x_sb[:, 1, sl],
                             start=False, stop=True)
            nc.scalar.activation(th_sb[:, sl], th_ps[:], AF.Identity,
                                 bias=bt_sb[:, 0:1])

            ph_ps = ps.tile([P, 512], F32, tag="s", name="ph_ps")
            nc.tensor.matmul(ph_ps[:], wpT_sb[:, 0], x_sb[:, 0, sl],
                             start=True, stop=False)
            nc.tensor.matmul(ph_ps[:], wpT_sb[:, 1], x_sb[:, 1, sl],
                             start=False, stop=True)
            nc.vector.tensor_scalar_add(ph_sb[:, sl], ph_ps[:], bp_sb[:, 0:1])

        # ---- g0 in (m, o) layout: lhsT = x column chunks ----
        for mc in range(MC):
            msl = slice(mc * P, (mc + 1) * P)
            g_ps = ps.tile([P, P], F32, tag="s", name="g_ps")
            nc.tensor.matmul(g_ps[:], x_sb[:, 0, msl], wgT_sb[:, 0],
                             start=True, stop=False)
            nc.tensor.matmul(g_ps[:], x_sb[:, 1, msl], wgT_sb[:, 1],
                             start=False, stop=True)
            nc.vector.tensor_copy(g_sb[:, mc], g_ps[:])

        # ---- attention main loop ----
        for q in range(NQn):
            qsl = slice(q * NQ, (q + 1) * NQ)
            y_ps = psy.tile([P, NQ], F32, tag="y", name="y_ps")
            # column-sum accumulator: PE ones-matmul path needs a persistent
            # PSUM tile; the all-DVE path only needs a transient for the
            # final partition-reduce, allocated later from the "s" rotation
            sum_ps = (psy.tile([P, NQ], F32, tag="sum", name="sum_ps")
                      if pe_mcs else None)
            accs = [None] * 4

            for mc in range(MC):
                msl = slice(mc * P, (mc + 1) * P)
                s_ps = ps.tile([P, NQ], F32, tag="s", name="s_ps")
                for b in range(NB):
                    bsl = slice(b * 512, (b + 1) * 512)
                    nc.tensor.matmul(
                        s_ps[:, bsl], ph_sb[:, msl],
                        th_sb[:, q * NQ + b * 512: q * NQ + (b + 1) * 512],
                        start=True, stop=True)
                exp_sb = work.tile([P, NQ], BF16, tag="exp", bufs=4,
                                   name="exp_sb")
                nc.scalar.activation(exp_sb[:], s_ps[:], AF.Exp,
                                     bias=cshift_sb[:, 0:1])

                for b in range(NB):
                    bsl = slice(b * 512, (b + 1) * 512)
                    nc.tensor.matmul(
                        y_ps[:, bsl], g_sb[:, mc], exp_sb[:, bsl],
                        start=(mc == 0), stop=(mc == MC - 1),
                        skip_group_check=True)

                if mc in pe_mcs:
                    last_pe = (mc == max(pe_mcs)) and len(pe_mcs) == MC
                    for b in range(NB):
                        bsl = slice(b * 512, (b + 1) * 512)
                        nc.tensor.matmul(
                            sum_ps[:, bsl], ones_sb[:], exp_sb[:, bsl],
                            start=(mc == min(pe_mcs)), stop=last_pe,
                            skip_group_check=True)
                else:
                    j = mc % 4
                    if accs[j] is None:
                        accs[j] = work.tile([P, NQ], BF16, tag=f"acc{j}",
                                            bufs=1, name=f"acc{j}_sb")
                        nc.vector.tensor_copy(accs[j][:], exp_sb[:])
                    else:
                        nc.vector.tensor_add(accs[j][:], accs[j][:],
                                             exp_sb[:])

            parts = [a for a in accs if a is not None]
            if parts:
                if sum_ps is None:
                    sum_ps = ps.tile([P, NQ], F32, tag="s", name="sumt_ps")
                # fold the bf16 partials into fp32 column sums on PE
                for pi, part in enumerate(parts):
                    for b in range(NB):
                        bsl = slice(b * 512, (b + 1) * 512)
                        nc.tensor.matmul(
                            sum_ps[:, bsl], ones_sb[:], part[:, bsl],
                            start=(len(pe_mcs) == 0 and pi == 0),
                            stop=(pi == len(parts) - 1),
                            skip_group_check=True)

            # 1/sums at ~18 bits via the custom-DVE fast reciprocal (the
            # exact `reciprocal` costs ~6 cycles/elem); sums are positive
            # and well inside its safe range
            recip_sb = work.tile([P, NQ], F32, tag="recip", name="recip_sb")
            nc.vector.reciprocal_approx_fast(recip_sb[:], sum_ps[:])
            yt_sb = work.tile([P, NQ], F32R, tag="yt", name="yt_sb")
            nc.vector.tensor_mul(yt_sb[:], y_ps[:], recip_sb[:])

            # out = wW @ yT + bW' + x
            for h in range(2):
                wy_ps = ps.tile([P, NQ], F32, tag="s", name="wy_ps")
                for b in range(NB):
                    bsl = slice(b * 512, (b + 1) * 512)
                    nc.tensor.matmul(
                        wy_ps[:, bsl], wWT_sb[:, h * P:(h + 1) * P],
                        yt_sb[:, bsl], start=True, stop=True)
                o_sb = work.tile([P, NQ], F32, tag="o", name="o_sb")
                nc.scalar.activation(o_sb[:], wy_ps[:], AF.Identity,
                                     bias=bWp_sb[:, h:h + 1])
                nc.vector.tensor_add(o_sb[:], o_sb[:], x_sb[:, h, qsl])
                nc.sync.dma_start(out_v[h, :, qsl], o_sb[:])

    nc.compile()
    return nc


_CACHE = {}


def _built(key=(N_FULL, 1024, 0)):
    if key not in _CACHE:
        _CACHE[key] = build_nc(*key)
    return _CACHE[key]


def make_in_maps(x, wg, bg, wt, bt, wp, bp, wW, bW):
    """Host-side prep: per-core input dicts (core b <- batch b)."""
    x = np.asarray(x, np.float32)
    B, C_, H, W = x.shape
    N = H * W
    xf = np.ascontiguousarray(x.reshape(B, C_, N))
    wg, bg, wt, bt, wp, bp, wW, bW = [
        np.asarray(a, np.float32) for a in (wg, bg, wt, bt, wp, bp, wW, bW)]
    def pack(w):  # (128, C) conv weight -> partition-major lhsT chunks
        return np.ascontiguousarray(
            w.T.reshape(2, P, P).transpose(1, 0, 2).reshape(P, 2 * P))

    wtT, wpT, wgT = pack(wt), pack(wp), pack(wg)
    wWT = np.ascontiguousarray(wW.T)                       # (128, 256)
    bWp = (wW @ bg + bW).astype(np.float32)                # fold bg into bW
    bWp = np.ascontiguousarray(bWp.reshape(2, P).T)        # (128, 2)
    shared = {
        "wtT": wtT, "wpT": wpT, "wgT": wgT, "wWT": wWT,
        "bt": bt.reshape(P, 1).copy(), "bp": bp.reshape(P, 1).copy(),
        "bWp": bWp,
    }
    return [{"x": np.ascontiguousarray(xf[b]), **shared} for b in range(B)]


def kernel(x, wg, bg, wt, bt, wp, bp, wW, bW):
    from concourse.bass_utils import run_bass_kernel_spmd

    B, C_, H, W = np.asarray(x).shape
    in_maps = make_in_maps(x, wg, bg, wt, bt, wp, bp, wW, bW)
    nc = _built()
    res = run_bass_kernel_spmd(nc, in_maps, core_ids=list(range(B)))
    out = np.stack([res.results[b]["out"] for b in range(B)])
    return out.reshape(B, C_, H, W).astype(np.float32)



# revision 2
# speedup vs baseline: 1.1237x; 1.1237x over previous
"""NonLocalBlock (single-head attention, N=HW=4096, d=128) on 8 trn2 cores.

Sharding: data-parallel over batch (B=8) — one batch element per NeuronCore.
Per core, the whole block runs out of SBUF:

  xf (256, 4096) -> theta_T = wt@xf + bt      (128, N)   [PE + bias on copy]
                    phi     = wp@xf           (128, N)   [PE; bp dropped - see below]
                    g0      = (wg@xf)^T       (N, 128)   [PE, no bias]
  S^T[m, n] = sum_i phi[i,m] * theta_T[i,n]   (keys m on partitions)
  expS = exp(S^T - 40)                         [ACT for most key-chunks,
                                                DVE Schraudolph bit-trick for some]
  sums[n] = sum_m expS[m, n]                   [DVE bf16 partial adds + PE ones-fold]
  yT[o, n] = (sum_m g0[m,o] expS[m,n]) / sums[n]
  out = wW @ yT + (wW@bg + bW) + xf

Numerics:
 - phi's bias bp is dropped: it contributes only a per-query additive term
   v[n] (+ const) to S, which softmax is invariant to.
 - Softmax without per-row max: scores are ~N(0, 128), |S| < ~91 empirically;
   exp(S - 40) stays in fp32/bf16 range (see baseline analysis).
 - theta/phi stored fp16 (stationary operand gets fast-weight-load; fp16
   mantissa keeps the score error ~3e-3 absolute, negligible through exp).
 - A subset of key-chunks per quarter computes exp on the Vector engine via
   the Schraudolph bit trick: bf16_bits(e^x) ~= rint(x*128*log2(e) + 16256),
   done as one tensor_scalar (fp32 PSUM -> uint16 SBUF, round+saturate) whose
   output is bitcast to bf16.  Max rel err ~3.3% on those chunks' weights;
   verified on HW (round-to-nearest + saturation at 0).  This offloads the
   otherwise ACT-bound exp stream.

Schedule: flat 128-step loop (4 query-quarters x 32 key-chunks), AV matmuls
skewed 2 chunks behind S matmuls for pipeline elasticity, quarter-tail
(fold/recip/normalize/Wy/out) software-pipelined into the next quarter's
first steps, projections (phi/g) interleaved with the streaming x DMA.
"""

import numpy as np
from contextlib import ExitStack

import concourse.bass as bass
import concourse.mybir as mybir
import concourse.tile as tile
from concourse import bacc

P = 128          # partitions / inter channels
C = 256          # input channels
F32 = mybir.dt.float32
F32R = mybir.dt.float32r
FP16 = mybir.dt.float16
U16 = mybir.dt.uint16
BF16 = mybir.dt.bfloat16
AF = mybir.ActivationFunctionType
ALU = mybir.AluOpType
CSHIFT = 40.0    # global score shift before exp (softmax-invariant)

LOG2E = 1.4426950408889634
SCHR_MUL = float(np.float32(128 * LOG2E))
SCHR_ADD = float(np.float32(16256 - CSHIFT * 128 * LOG2E - 5.61))

B_FULL = 8
H_FULL = 64
W_FULL = 64
N_FULL = H_FULL * W_FULL

NQ = 1024                     # query-quarter width
# key-chunks (of 32 per quarter) whose exp runs on DVE instead of ACT
DVE_EXP_MCS = (5, 13, 21, 27)


def build_nc(N=N_FULL):
    MC = N // P                   # 32 key chunks
    NQn = N // NQ                 # 4 query quarters
    NB = NQ // 512                # 2 512-wide blocks per quarter

    nc = bacc.Bacc("TRN2", target_bir_lowering=False, debug=False)

    x_d = nc.dram_tensor("x", [C, N], F32R, kind="ExternalInput").ap()
    wtT_d = nc.dram_tensor("wtT", [P, 2 * P], F32R, kind="ExternalInput").ap()
    wpT_d = nc.dram_tensor("wpT", [P, 2 * P], F32R, kind="ExternalInput").ap()
    wgT_d = nc.dram_tensor("wgT", [P, 2 * P], F32R, kind="ExternalInput").ap()
    wWT_d = nc.dram_tensor("wWT", [P, C], F32R, kind="ExternalInput").ap()
    bt_d = nc.dram_tensor("bt", [P, 1], F32, kind="ExternalInput").ap()
    bWp_d = nc.dram_tensor("bWp", [P, 2], F32, kind="ExternalInput").ap()
    out_d = nc.dram_tensor("out", [C, N], F32, kind="ExternalOutput").ap()

    x_v = x_d.rearrange("(k p) n -> k p n", p=P)
    out_v = out_d.rearrange("(k p) n -> k p n", p=P)

    with tile.TileContext(nc) as tc, ExitStack() as ctx:
        const = ctx.enter_context(tc.tile_pool(name="const", bufs=1))
        big = ctx.enter_context(tc.tile_pool(name="big", bufs=1))
        work = ctx.enter_context(tc.tile_pool(name="work", bufs=3))
        ps = ctx.enter_context(tc.tile_pool(name="ps", bufs=3, space="PSUM"))
        psy = ctx.enter_context(tc.tile_pool(name="psy", bufs=1, space="PSUM"))

        # ---- constants ----
        wtT_sb = const.tile([P, 2, P], F32R, name="wtT_sb")
        wpT_sb = const.tile([P, 2, P], F32R, name="wpT_sb")
        wgT_sb = const.tile([P, 2, P], F32R, name="wgT_sb")
        wWT_sb = const.tile([P, C], F32R, name="wWT_sb")
        bt_sb = const.tile([P, 1], F32, name="bt_sb")
        bWp_sb = const.tile([P, 2], F32, name="bWp_sb")
        ones_sb = const.tile([P, P], BF16, name="ones_sb")
        cshift_sb = const.tile([P, 1], F32, name="cshift_sb")
        nc.vector.memset(cshift_sb[:], -CSHIFT)
        nc.vector.memset(ones_sb[:], 1.0)

        nc.sync.dma_start(wtT_sb[:], wtT_d.rearrange("p (k i) -> p k i", k=2))
        nc.sync.dma_start(wpT_sb[:], wpT_d.rearrange("p (k i) -> p k i", k=2))
        nc.sync.dma_start(wgT_sb[:], wgT_d.rearrange("p (k i) -> p k i", k=2))
        nc.sync.dma_start(wWT_sb[:], wWT_d)
        nc.sync.dma_start(bt_sb[:], bt_d)
        nc.sync.dma_start(bWp_sb[:], bWp_d)

        # ---- x load, block-major so both halves of early blocks land first
        x_sb = big.tile([P, 2, N], F32R, name="x_sb")
        for blk in range(N // 512):
            for k in range(2):
                nc.sync.dma_start(
                    x_sb[:, k, blk * 512:(blk + 1) * 512],
                    x_v[k, :, blk * 512:(blk + 1) * 512],
                )

        th_sb = big.tile([P, N], FP16, name="th_sb")   # theta^T (i, n)
        ph_sb = big.tile([P, N], FP16, name="ph_sb")   # phi (i, m)
        g_sb = big.tile([P, MC, P], BF16, name="g_sb")  # g0 (m_in, chunk, o)

        def th_block(b):          # theta columns [512b, 512b+512)
            sl = slice(b * 512, (b + 1) * 512)
            th_ps = ps.tile([P, NQ], F32, tag="s", name="th_ps")
            nc.tensor.matmul(th_ps[:, 0:512], wtT_sb[:, 0], x_sb[:, 0, sl],
                             start=True, stop=False)
            nc.tensor.matmul(th_ps[:, 0:512], wtT_sb[:, 1], x_sb[:, 1, sl],
                             start=False, stop=True)
            nc.scalar.activation(th_sb[:, sl], th_ps[:, 0:512], AF.Identity,
                                 bias=bt_sb[:, 0:1])

        def ph_block(b):          # phi columns (keys) [512b, 512b+512)
            sl = slice(b * 512, (b + 1) * 512)
            ph_ps = ps.tile([P, NQ], F32, tag="s", name="ph_ps")
            nc.tensor.matmul(ph_ps[:, 0:512], wpT_sb[:, 0], x_sb[:, 0, sl],
                             start=True, stop=False)
            nc.tensor.matmul(ph_ps[:, 0:512], wpT_sb[:, 1], x_sb[:, 1, sl],
                             start=False, stop=True)
            nc.scalar.copy(ph_sb[:, sl], ph_ps[:, 0:512])

        def g_chunk(mc):          # g0 rows (keys) [128mc, 128mc+128)
            msl = slice(mc * P, (mc + 1) * P)
            g_ps = ps.tile([P, NQ], F32, tag="s", name="g_ps")
            nc.tensor.matmul(g_ps[:, 0:P], x_sb[:, 0, msl], wgT_sb[:, 0],
                             start=True, stop=False)
            nc.tensor.matmul(g_ps[:, 0:P], x_sb[:, 1, msl], wgT_sb[:, 1],
                             start=False, stop=True)
            if mc % 2 == 0:
                nc.vector.tensor_copy(g_sb[:, mc], g_ps[:, 0:P])
            else:
                nc.scalar.copy(g_sb[:, mc], g_ps[:, 0:P])

        # per-quarter state carried across the flat loop
        state = {}

        def start_quarter(q):
            state[q] = {
                "y": psy.tile([P, NQ], F32, tag="y", name=f"y{q}_ps"),
                "acc": [None, None],
                "exp": [None] * MC,
            }

        def emit_S(q, mc):
            st = state[q]
            msl = slice(mc * P, (mc + 1) * P)
            s_ps = ps.tile([P, NQ], F32, tag="s", name="s_ps")
            for b in range(NB):
                nc.tensor.matmul(
                    s_ps[:, b * 512:(b + 1) * 512], ph_sb[:, msl],
                    th_sb[:, q * NQ + b * 512: q * NQ + (b + 1) * 512],
                    start=True, stop=True)
            exp_t = work.tile([P, NQ], BF16, tag="exp", bufs=5, name="exp_sb")
            if mc in DVE_EXP_MCS:
                nc.vector.tensor_scalar(
                    exp_t[:].bitcast(U16), s_ps[:], SCHR_MUL, SCHR_ADD,
                    ALU.mult, ALU.add)
            else:
                nc.scalar.activation(exp_t[:], s_ps[:], AF.Exp,
                                     bias=cshift_sb[:, 0:1])
            st["exp"][mc] = exp_t

        def emit_AV(q, mc):
            st = state[q]
            exp_t = st["exp"][mc]
            for b in range(NB):
                bsl = slice(b * 512, (b + 1) * 512)
                nc.tensor.matmul(
                    st["y"][:, bsl], g_sb[:, mc], exp_t[:, bsl],
                    start=(mc == 0), stop=(mc == MC - 1),
                    skip_group_check=True)
            j = mc % 2
            if st["acc"][j] is None:
                st["acc"][j] = work.tile([P, NQ], BF16, tag=f"acc{j}",
                                         bufs=2, name=f"acc{j}_sb")
                nc.vector.tensor_copy(st["acc"][j][:], exp_t[:])
            else:
                nc.vector.tensor_add(st["acc"][j][:], st["acc"][j][:],
                                     exp_t[:])
            st["exp"][mc] = None

        def emit_fold(q):
            st = state[q]
            sumt = ps.tile([P, NQ], F32, tag="s", name="sumt_ps")
            for b in range(NB):
                bsl = slice(b * 512, (b + 1) * 512)
                for j in range(2):
                    nc.tensor.matmul(sumt[:, bsl], ones_sb[:],
                                     st["acc"][j][:, bsl],
                                     start=(j == 0), stop=(j == 1),
                                     skip_group_check=True)
            st["sumt"] = sumt

        def emit_norm(q):
            st = state[q]
            recip = work.tile([P, NQ], F32, tag="recip", bufs=2,
                              name="recip_sb")
            yt = work.tile([P, NQ], F32R, tag="yt", bufs=2, name="yt_sb")
            for b in range(NB):
                bsl = slice(b * 512, (b + 1) * 512)
                nc.vector.reciprocal_approx_fast(recip[:, bsl],
                                                 st["sumt"][:, bsl])
                nc.vector.tensor_mul(yt[:, bsl], st["y"][:, bsl],
                                     recip[:, bsl])
            st["yt"] = yt

        def emit_out(q):
            st = state[q]
            qsl0 = q * NQ
            for h in range(2):
                wy_ps = ps.tile([P, NQ], F32, tag="s", name="wy_ps")
                for b in range(NB):
                    bsl = slice(b * 512, (b + 1) * 512)
                    nc.tensor.matmul(
                        wy_ps[:, bsl], wWT_sb[:, h * P:(h + 1) * P],
                        st["yt"][:, bsl], start=True, stop=True)
                o_sb = work.tile([P, NQ], F32, tag="o", bufs=3, name="o_sb")
                for b in range(NB):
                    bsl = slice(b * 512, (b + 1) * 512)
                    csl = slice(qsl0 + b * 512, qsl0 + (b + 1) * 512)
                    nc.scalar.activation(o_sb[:, bsl], wy_ps[:, bsl],
                                         AF.Identity, bias=bWp_sb[:, h:h + 1])
                    nc.vector.tensor_add(o_sb[:, bsl], o_sb[:, bsl],
                                         x_sb[:, h, csl])
                    nc.sync.dma_start(out_v[h, :, csl], o_sb[:, bsl])

        def finish_quarter(q):
            # flush the 2-chunk AV skew, then fold the denominator partials
            emit_AV(q, MC - 2)
            emit_AV(q, MC - 1)
            emit_fold(q)

        # ---- emission ----
        th_block(0)
        th_block(1)

        for t in range(NQn * MC):
            q, mc = divmod(t, MC)
            if mc == 0:
                if q == 0:
                    start_quarter(0)
                else:
                    finish_quarter(q - 1)
                    start_quarter(q)
            if q == 0 and mc % 4 == 0:
                ph_block(mc // 4)
                for m2 in range(mc, mc + 4):
                    g_chunk(m2)
            if q > 0:
                if mc == 1:
                    emit_norm(q - 1)
                elif mc == 2:
                    emit_out(q - 1)
                    del state[q - 1]
            if q < NQn - 1:
                if mc == 8:
                    th_block(2 * (q + 1))
                elif mc == 16:
                    th_block(2 * (q + 1) + 1)
            emit_S(q, mc)
            if mc >= 2:
                emit_AV(q, mc - 2)

        finish_quarter(NQn - 1)
        emit_norm(NQn - 1)
        emit_out(NQn - 1)

    nc.compile()
    return nc


_CACHE = {}


def _built(key=(N_FULL,)):
    if key not in _CACHE:
        _CACHE[key] = build_nc(*key)
    return _CACHE[key]


def make_in_maps(x, wg, bg, wt, bt, wp, bp, wW, bW):
    """Host-side prep: per-core input dicts (core b <- batch b)."""
    x = np.asarray(x, np.float32)
    B, C_, H, W = x.shape
    N = H * W
    xf = np.ascontiguousarray(x.reshape(B, C_, N))
    wg, bg, wt, bt, wp, bp, wW, bW = [
        np.asarray(a, np.float32) for a in (wg, bg, wt, bt, wp, bp, wW, bW)]

    def pack(w):  # (128, C) conv weight -> partition-major lhsT chunks
        return np.ascontiguousarray(
            w.T.reshape(2, P, P).transpose(1, 0, 2).reshape(P, 2 * P))

    wtT, wpT, wgT = pack(wt), pack(wp), pack(wg)
    wWT = np.ascontiguousarray(wW.T)                       # (128, 256)
    bWp = (wW @ bg + bW).astype(np.float32)                # fold bg into bW
    bWp = np.ascontiguousarray(bWp.reshape(2, P).T)        # (128, 2)
    shared = {
        "wtT": wtT, "wpT": wpT, "wgT": wgT, "wWT": wWT,
        "bt": bt.reshape(P, 1).copy(), "bWp": bWp,
    }
    return [{"x": np.ascontiguousarray(xf[b]), **shared} for b in range(B)]


def kernel(x, wg, bg, wt, bt, wp, bp, wW, bW):
    from concourse.bass_utils import run_bass_kernel_spmd

    B, C_, H, W = np.asarray(x).shape
    in_maps = make_in_maps(x, wg, bg, wt, bt, wp, bp, wW, bW)
    nc = _built()
    res = run_bass_kernel_spmd(nc, in_maps, core_ids=list(range(B)))
    out = np.stack([res.results[b]["out"] for b in range(B)])
    return out.reshape(B, C_, H, W).astype(np.float32)


# revision 5
# speedup vs baseline: 1.1336x; 1.0088x over previous
"""NonLocalBlock (single-head attention, N=HW=4096, d=128) on 8 trn2 cores.

Sharding: data-parallel over batch (B=8) — one batch element per NeuronCore.
Per core, the whole block runs out of SBUF:

  xf (256, 4096) -> theta_T = wt@xf + bt      (128, N)   [PE + bias on copy]
                    phi     = wp@xf           (128, N)   [PE; bp dropped]
                    gT      = wg@xf           (128, N)   [PE]
                    g0      = gT^T chunks     (N, 128)   [xbar DMA transpose]
  S^T[m, n] = sum_i phi[i,m] * theta_T[i,n]   (keys m on partitions)
  expS = exp(S^T - 40)                         [ACT, some chunks DVE bit-trick]
  sums[n] = sum_m expS[m, n]                   [DVE bf16 partial adds + PE fold]
  yT[o, n] = (sum_m g0[m,o] expS[m,n]) / sums[n]
  out = wW @ yT + (wW@bg + bW) + xf

Numerics:
 - phi's bias bp only adds a per-query constant to S -> softmax-invariant,
   dropped entirely.
 - No per-row max: scores ~N(0,128), exp(S-40) stays in range (see analysis).
 - theta/phi stored fp16: stationary operand gets FWL (2x faster weight
   load); fp16 keeps the absolute score error ~3e-3 (negligible through exp).
 - DVE_EXP_MCS key-chunks per quarter compute exp on the Vector engine via
   the Schraudolph bit trick: bf16_bits(e^x) ~= rint(x*128*log2e + 16256) as
   one tensor_scalar (fp32 PSUM -> uint16, HW round-to-nearest + saturate),
   bitcast to bf16.  ~3.3% max rel err on those chunks' weights; offloads
   the otherwise-bottleneck ACT exp stream.

Schedule: flat 128-step loop (4 query-quarters x 32 key-chunks).  AV matmuls
run 2 chunks behind S for elasticity; each quarter's tail (fold / reciprocal
/ normalize / Wy / +x / store) is emitted inside the next quarter's first
steps in 512-column blocks so no engine drains; projections stream behind
the x DMA; dummy PE warmup keeps HAM from throttling the prologue.
"""

import numpy as np
from contextlib import ExitStack

import concourse.bass as bass
import concourse.mybir as mybir
import concourse.tile as tile
from concourse import bacc

P = 128          # partitions / inter channels
C = 256          # input channels
F32 = mybir.dt.float32
F32R = mybir.dt.float32r
FP16 = mybir.dt.float16
U16 = mybir.dt.uint16
BF16 = mybir.dt.bfloat16
AF = mybir.ActivationFunctionType
ALU = mybir.AluOpType
CSHIFT = 40.0    # global score shift before exp (softmax-invariant)

LOG2E = 1.4426950408889634
SCHR_MUL = float(np.float32(128 * LOG2E))
SCHR_ADD = float(np.float32(16256 - CSHIFT * 128 * LOG2E - 5.61))

B_FULL = 8
H_FULL = 64
W_FULL = 64
N_FULL = H_FULL * W_FULL

NQ = 1024                     # query-quarter width
# key-chunks (of 32 per quarter) whose exp runs on DVE instead of ACT
DVE_EXP_MCS = (4, 9, 14, 19, 23, 27)
WARMUP_MMS = 44               # ~4us of cold 128-col matmuls to flip HAM
G_VIA_TRANSPOSE = False       # gT + xbar transpose vs per-chunk matmuls


def build_nc(N=N_FULL):
    MC = N // P                   # 32 key chunks
    NQn = N // NQ                 # 4 query quarters
    NB = NQ // 512                # 2 512-wide blocks per quarter

    nc = bacc.Bacc("TRN2", target_bir_lowering=False, debug=False)

    x_d = nc.dram_tensor("x", [C, N], F32R, kind="ExternalInput").ap()
    wtT_d = nc.dram_tensor("wtT", [P, 2 * P], F32R, kind="ExternalInput").ap()
    wpT_d = nc.dram_tensor("wpT", [P, 2 * P], F32R, kind="ExternalInput").ap()
    wgT_d = nc.dram_tensor("wgT", [P, 2 * P], F32R, kind="ExternalInput").ap()
    wWT_d = nc.dram_tensor("wWT", [P, C], F32R, kind="ExternalInput").ap()
    bt_d = nc.dram_tensor("bt", [P, 1], F32, kind="ExternalInput").ap()
    bWp_d = nc.dram_tensor("bWp", [P, 2], F32, kind="ExternalInput").ap()
    out_d = nc.dram_tensor("out", [C, N], F32, kind="ExternalOutput").ap()

    x_v = x_d.rearrange("(k p) n -> k p n", p=P)
    out_v = out_d.rearrange("(k p) n -> k p n", p=P)

    with tile.TileContext(nc) as tc, ExitStack() as ctx:
        const = ctx.enter_context(tc.tile_pool(name="const", bufs=1))
        big = ctx.enter_context(tc.tile_pool(name="big", bufs=1))
        work = ctx.enter_context(tc.tile_pool(name="work", bufs=3))
        ps = ctx.enter_context(tc.tile_pool(name="ps", bufs=3, space="PSUM"))
        psy = ctx.enter_context(tc.tile_pool(name="psy", bufs=1, space="PSUM"))

        # ---- constants ----
        wtT_sb = const.tile([P, 2, P], F32R, name="wtT_sb")
        wpT_sb = const.tile([P, 2, P], F32R, name="wpT_sb")
        wgT_sb = const.tile([P, 2, P], F32R, name="wgT_sb")
        wWT_sb = const.tile([P, C], F32R, name="wWT_sb")
        bt_sb = const.tile([P, 1], F32, name="bt_sb")
        bWp_sb = const.tile([P, 2], F32, name="bWp_sb")
        ones_sb = const.tile([P, P], BF16, name="ones_sb")
        cshift_sb = const.tile([P, 1], F32, name="cshift_sb")
        nc.vector.memset(cshift_sb[:], -CSHIFT)
        nc.vector.memset(ones_sb[:], 1.0)

        nc.sync.dma_start(wtT_sb[:], wtT_d.rearrange("p (k i) -> p k i", k=2))
        nc.sync.dma_start(wpT_sb[:], wpT_d.rearrange("p (k i) -> p k i", k=2))
        nc.sync.dma_start(wgT_sb[:], wgT_d.rearrange("p (k i) -> p k i", k=2))
        nc.sync.dma_start(wWT_sb[:], wWT_d)
        nc.sync.dma_start(bt_sb[:], bt_d)
        nc.sync.dma_start(bWp_sb[:], bWp_d)

        # ---- PE warmup: keep the HAM activity window busy while the x DMA
        # streams in, so real matmuls start at 2.4 GHz instead of 1.2.
        warm_ps = ps.tile([P, NQ], F32, tag="s", name="warm_ps")
        for _ in range(WARMUP_MMS):
            nc.tensor.matmul(warm_ps[:, 0:P], ones_sb[:], ones_sb[:],
                             start=True, stop=True, skip_group_check=True)

        x_sb = big.tile([P, 2, N], F32R, name="x_sb")

        def x_dma(b):
            for k in range(2):
                nc.sync.dma_start(
                    x_sb[:, k, b * 512:(b + 1) * 512],
                    x_v[k, :, b * 512:(b + 1) * 512],
                )

        for b in range(3):        # blocks 3.. streamed inside the q0 loop
            x_dma(b)

        th_sb = big.tile([P, N], FP16, name="th_sb")   # theta^T (i, n)
        ph_sb = big.tile([P, N], FP16, name="ph_sb")   # phi (i, m)
        gT_sb = big.tile([P, N], BF16, name="gT_sb")   # g0^T (o, m)
        g_sb = big.tile([P, MC, P], BF16, name="g_sb")  # g0 (m_in, chunk, o)

        def proj_block(b, wT, dst, bias):
            sl = slice(b * 512, (b + 1) * 512)
            p_ps = ps.tile([P, NQ], F32, tag="s", name="p_ps")
            nc.tensor.matmul(p_ps[:, 0:512], wT[:, 0], x_sb[:, 0, sl],
                             start=True, stop=False)
            nc.tensor.matmul(p_ps[:, 0:512], wT[:, 1], x_sb[:, 1, sl],
                             start=False, stop=True)
            if bias is None:
                nc.scalar.copy(dst[:, sl], p_ps[:, 0:512])
            else:
                nc.scalar.activation(dst[:, sl], p_ps[:, 0:512], AF.Identity,
                                     bias=bias)

        # per-quarter state carried across the flat loop
        state = {}

        def start_quarter(q):
            state[q] = {
                "y": psy.tile([P, NQ], F32, tag="y", name=f"y{q}_ps"),
                "acc": [None, None],
                "exp": [None] * MC,
            }

        def emit_S(q, mc):
            st = state[q]
            msl = slice(mc * P, (mc + 1) * P)
            s_ps = ps.tile([P, NQ], F32, tag="s", name="s_ps")
            for b in range(NB):
                nc.tensor.matmul(
                    s_ps[:, b * 512:(b + 1) * 512], ph_sb[:, msl],
                    th_sb[:, q * NQ + b * 512: q * NQ + (b + 1) * 512],
                    start=True, stop=True)
            exp_t = work.tile([P, NQ], BF16, tag="exp", bufs=6, name="exp_sb")
            if mc in DVE_EXP_MCS:
                nc.vector.tensor_scalar(
                    exp_t[:].bitcast(U16), s_ps[:], SCHR_MUL, SCHR_ADD,
                    ALU.mult, ALU.add)
            else:
                nc.scalar.activation(exp_t[:], s_ps[:], AF.Exp,
                                     bias=cshift_sb[:, 0:1])
            st["exp"][mc] = exp_t

        def emit_AV(q, mc):
            st = state[q]
            exp_t = st["exp"][mc]
            for b in range(NB):
                bsl = slice(b * 512, (b + 1) * 512)
                nc.tensor.matmul(
                    st["y"][:, bsl], g_sb[:, mc], exp_t[:, bsl],
                    start=(mc == 0), stop=(mc == MC - 1),
                    skip_group_check=True)
            j = mc % 2
            if st["acc"][j] is None:
                st["acc"][j] = work.tile([P, NQ], BF16, tag=f"acc{j}",
                                         bufs=2, name=f"acc{j}_sb")
                nc.vector.tensor_copy(st["acc"][j][:], exp_t[:])
            else:
                nc.vector.tensor_add(st["acc"][j][:], st["acc"][j][:],
                                     exp_t[:])
            st["exp"][mc] = None

        def finish_quarter(q):
            # flush the 2-chunk AV skew, then fold the denominator partials
            emit_AV(q, MC - 2)
            emit_AV(q, MC - 1)
            st = state[q]
            sumt = ps.tile([P, NQ], F32, tag="s", name="sumt_ps")
            for b in range(NB):
                bsl = slice(b * 512, (b + 1) * 512)
                for j in range(2):
                    nc.tensor.matmul(sumt[:, bsl], ones_sb[:],
                                     st["acc"][j][:, bsl],
                                     start=(j == 0), stop=(j == 1),
                                     skip_group_check=True)
            st["sumt"] = sumt

        def emit_tail_block(q, b):
            """Normalize + project + store one 512-col block of quarter q."""
            st = state[q]
            bsl = slice(b * 512, (b + 1) * 512)
            if b == 0:
                st["recip"] = work.tile([P, NQ], F32, tag="recip", bufs=2,
                                        name="recip_sb")
                st["yt"] = work.tile([P, NQ], F32R, tag="yt", bufs=2,
                                     name="yt_sb")
                st["wy"] = [ps.tile([P, NQ], F32, tag="s", name=f"wy{h}_ps")
                            for h in range(2)]
                st["o"] = [work.tile([P, NQ], F32, tag=f"o{h}", bufs=2,
                                     name=f"o{h}_sb") for h in range(2)]
            nc.vector.reciprocal_approx_fast(st["recip"][:, bsl],
                                             st["sumt"][:, bsl])
            nc.vector.tensor_mul(st["yt"][:, bsl], st["y"][:, bsl],
                                 st["recip"][:, bsl])
            csl = slice(q * NQ + b * 512, q * NQ + (b + 1) * 512)
            for h in range(2):
                nc.tensor.matmul(st["wy"][h][:, bsl],
                                 wWT_sb[:, h * P:(h + 1) * P],
                                 st["yt"][:, bsl], start=True, stop=True)
            for h in range(2):
                nc.scalar.activation(st["o"][h][:, bsl], st["wy"][h][:, bsl],
                                     AF.Identity, bias=bWp_sb[:, h:h + 1])
                nc.vector.tensor_add(st["o"][h][:, bsl], st["o"][h][:, bsl],
                                     x_sb[:, h, csl])
                nc.sync.dma_start(out_v[h, :, csl], st["o"][h][:, bsl])

        # ---- emission ----
        th_ready = 0

        def emit_th(b):
            proj_block(b, wtT_sb, th_sb, bt_sb[:, 0:1])

        emit_th(0)
        emit_th(1)

        for t in range(NQn * MC):
            q, mc = divmod(t, MC)
            if mc == 0:
                start_quarter(q)
            if q == 0 and mc % 4 == 0:
                b = mc // 4
                if b + 3 <= 7:
                    x_dma(b + 3)
                proj_block(b, wpT_sb, ph_sb, None)     # phi keys block
                if G_VIA_TRANSPOSE:
                    proj_block(b, wgT_sb, gT_sb, None)  # gT keys block
                    for m2 in range(4 * b, 4 * b + 4):  # g chunks via xbar
                        nc.sync.dma_start_transpose(
                            g_sb[:, m2], gT_sb[:, m2 * P:(m2 + 1) * P])
                else:
                    for m2 in range(4 * b, 4 * b + 4):
                        g_ps = ps.tile([P, NQ], F32, tag="s", name="g_ps")
                        msl = slice(m2 * P, (m2 + 1) * P)
                        nc.tensor.matmul(g_ps[:, 0:P], x_sb[:, 0, msl],
                                         wgT_sb[:, 0], start=True, stop=False)
                        nc.tensor.matmul(g_ps[:, 0:P], x_sb[:, 1, msl],
                                         wgT_sb[:, 1], start=False, stop=True)
                        if m2 % 2 == 0:
                            nc.vector.tensor_copy(g_sb[:, m2], g_ps[:, 0:P])
                        else:
                            nc.scalar.copy(g_sb[:, m2], g_ps[:, 0:P])
            if q < NQn - 1 and mc in (8, 16):
                emit_th(2 * (q + 1) + (mc == 16))
            emit_S(q, mc)
            if q > 0:
                if mc == 1:
                    finish_quarter(q - 1)
                elif mc == 2:
                    emit_tail_block(q - 1, 0)
                elif mc == 3:
                    emit_tail_block(q - 1, 1)
                    del state[q - 1]
            # AV skew: 2 behind normally; first AVs of q>0 wait for the
            # previous quarter's y_ps to be fully consumed (psy bufs=1)
            if q == 0:
                if mc >= 2:
                    emit_AV(q, mc - 2)
            else:
                if mc == 4:
                    emit_AV(q, 0)
                    emit_AV(q, 1)
                    emit_AV(q, 2)
                elif mc >= 5:
                    emit_AV(q, mc - 2)

        finish_quarter(NQn - 1)
        emit_tail_block(NQn - 1, 0)
        emit_tail_block(NQn - 1, 1)

    nc.compile()
    return nc


_CACHE = {}


def _built(key=(N_FULL,)):
    if key not in _CACHE:
        _CACHE[key] = build_nc(*key)
    return _CACHE[key]


def make_in_maps(x, wg, bg, wt, bt, wp, bp, wW, bW):
    """Host-side prep: per-core input dicts (core b <- batch b)."""
    x = np.asarray(x, np.float32)
    B, C_, H, W = x.shape
    N = H * W
    xf = np.ascontiguousarray(x.reshape(B, C_, N))
    wg, bg, wt, bt, wp, bp, wW, bW = [
        np.asarray(a, np.float32) for a in (wg, bg, wt, bt, wp, bp, wW, bW)]

    def pack(w):  # (128, C) conv weight -> partition-major lhsT chunks
        return np.ascontiguousarray(
            w.T.reshape(2, P, P).transpose(1, 0, 2).reshape(P, 2 * P))

    wtT, wpT, wgT = pack(wt), pack(wp), pack(wg)
    wWT = np.ascontiguousarray(wW.T)                       # (128, 256)
    bWp = (wW @ bg + bW).astype(np.float32)                # fold bg into bW
    bWp = np.ascontiguousarray(bWp.reshape(2, P).T)        # (128, 2)
    shared = {
        "wtT": wtT, "wpT": wpT, "wgT": wgT, "wWT": wWT,
        "bt": bt.reshape(P, 1).copy(), "bWp": bWp,
    }
    return [{"x": np.ascontiguousarray(xf[b]), **shared} for b in range(B)]


def kernel(x, wg, bg, wt, bt, wp, bp, wW, bW):
    from concourse.bass_utils import run_bass_kernel_spmd

    B, C_, H, W = np.asarray(x).shape
    in_maps = make_in_maps(x, wg, bg, wt, bt, wp, bp, wW, bW)
    nc = _built()
    res = run_bass_kernel_spmd(nc, in_maps, core_ids=list(range(B)))
    out = np.stack([res.results[b]["out"] for b in range(B)])
    return out.reshape(B, C_, H, W).astype(np.float32)


# revision 10
# speedup vs baseline: 1.1396x; 1.0053x over previous
"""NonLocalBlock (single-head attention, N=HW=4096, d=128) on 8 trn2 cores.

Sharding: data-parallel over batch (B=8) — one batch element per NeuronCore.
Per core, the whole block runs out of SBUF:

  xf (256, 4096) -> theta_T = wt@xf + bt      (128, N)   [PE + bias on copy]
                    phi     = wp@xf           (128, N)   [PE; bp dropped]
                    gT      = wg@xf           (128, N)   [PE]
                    g0      = gT^T chunks     (N, 128)   [xbar DMA transpose]
  S^T[m, n] = sum_i phi[i,m] * theta_T[i,n]   (keys m on partitions)
  expS = exp(S^T - 40)                         [ACT, some chunks DVE bit-trick]
  sums[n] = sum_m expS[m, n]                   [DVE bf16 partial adds + PE fold]
  yT[o, n] = (sum_m g0[m,o] expS[m,n]) / sums[n]
  out = wW @ yT + (wW@bg + bW) + xf

Numerics:
 - phi's bias bp only adds a per-query constant to S -> softmax-invariant,
   dropped entirely.
 - No per-row max: scores ~N(0,128), exp(S-40) stays in range (see analysis).
 - theta/phi stored fp16: stationary operand gets FWL (2x faster weight
   load); fp16 keeps the absolute score error ~3e-3 (negligible through exp).
 - DVE_EXP_MCS key-chunks per quarter compute exp on the Vector engine via
   the Schraudolph bit trick: bf16_bits(e^x) ~= rint(x*128*log2e + 16256) as
   one tensor_scalar (fp32 PSUM -> uint16, HW round-to-nearest + saturate),
   bitcast to bf16.  ~3.3% max rel err on those chunks' weights; offloads
   the otherwise-bottleneck ACT exp stream.

Schedule: flat 128-step loop (4 query-quarters x 32 key-chunks).  AV matmuls
run 2 chunks behind S for elasticity; each quarter's tail (fold / reciprocal
/ normalize / Wy / +x / store) is emitted inside the next quarter's first
steps in 512-column blocks so no engine drains; projections stream behind
the x DMA; dummy PE warmup keeps HAM from throttling the prologue.
"""

import numpy as np
from contextlib import ExitStack

import concourse.bass as bass
import concourse.mybir as mybir
import concourse.tile as tile
from concourse import bacc

P = 128          # partitions / inter channels
C = 256          # input channels
F32 = mybir.dt.float32
F32R = mybir.dt.float32r
FP16 = mybir.dt.float16
U16 = mybir.dt.uint16
BF16 = mybir.dt.bfloat16
AF = mybir.ActivationFunctionType
ALU = mybir.AluOpType
CSHIFT = 40.0    # global score shift before exp (softmax-invariant)

LOG2E = 1.4426950408889634
SCHR_MUL = float(np.float32(128 * LOG2E))
SCHR_ADD = float(np.float32(16256 - CSHIFT * 128 * LOG2E - 5.61))

B_FULL = 8
H_FULL = 64
W_FULL = 64
N_FULL = H_FULL * W_FULL

NQ = 1024                     # query-quarter width
# key-chunks (of 32 per quarter) whose exp runs on DVE instead of ACT
DVE_EXP_MCS = (4, 11, 17, 23, 27)
WARMUP_MMS = 12               # keep PE ticking until the first x block lands
G_VIA_TRANSPOSE = False       # gT + xbar transpose vs per-chunk matmuls


def build_nc(N=N_FULL):
    MC = N // P                   # 32 key chunks
    NQn = N // NQ                 # 4 query quarters
    NB = NQ // 512                # 2 512-wide blocks per quarter

    nc = bacc.Bacc("TRN2", target_bir_lowering=False, debug=False)

    x_d = nc.dram_tensor("x", [C, N], F32R, kind="ExternalInput").ap()
    wtT_d = nc.dram_tensor("wtT", [P, 2 * P], F32R, kind="ExternalInput").ap()
    wpT_d = nc.dram_tensor("wpT", [P, 2 * P], F32R, kind="ExternalInput").ap()
    wgT_d = nc.dram_tensor("wgT", [P, 2 * P], F32R, kind="ExternalInput").ap()
    wWT_d = nc.dram_tensor("wWT", [P, C], F32R, kind="ExternalInput").ap()
    bt_d = nc.dram_tensor("bt", [P, 1], F32, kind="ExternalInput").ap()
    bWp_d = nc.dram_tensor("bWp", [P, 2], F32, kind="ExternalInput").ap()
    out_d = nc.dram_tensor("out", [C, N], F32, kind="ExternalOutput").ap()

    x_v = x_d.rearrange("(k p) n -> k p n", p=P)
    out_v = out_d.rearrange("(k p) n -> k p n", p=P)

    with tile.TileContext(nc) as tc, ExitStack() as ctx:
        const = ctx.enter_context(tc.tile_pool(name="const", bufs=1))
        big = ctx.enter_context(tc.tile_pool(name="big", bufs=1))
        work = ctx.enter_context(tc.tile_pool(name="work", bufs=3))
        ps = ctx.enter_context(tc.tile_pool(name="ps", bufs=3, space="PSUM"))
        psy = ctx.enter_context(tc.tile_pool(name="psy", bufs=1, space="PSUM"))

        # ---- constants ----
        wtT_sb = const.tile([P, 2, P], F32R, name="wtT_sb")
        wpT_sb = const.tile([P, 2, P], F32R, name="wpT_sb")
        wgT_sb = const.tile([P, 2, P], F32R, name="wgT_sb")
        wWT_sb = const.tile([P, C], F32R, name="wWT_sb")
        bt_sb = const.tile([P, 1], F32, name="bt_sb")
        bWp_sb = const.tile([P, 2], F32, name="bWp_sb")
        ones_sb = const.tile([P, P], BF16, name="ones_sb")
        cshift_sb = const.tile([P, 1], F32, name="cshift_sb")
        nc.vector.memset(cshift_sb[:], -CSHIFT)
        nc.vector.memset(ones_sb[:], 1.0)

        x_sb = big.tile([P, 2, N], F32R, name="x_sb")

        def x_dma(b):
            for k in range(2):
                nc.sync.dma_start(
                    x_sb[:, k, b * 512:(b + 1) * 512],
                    x_v[k, :, b * 512:(b + 1) * 512],
                )

        # first x blocks ahead of the (slow-to-trigger) weight descriptors
        x_dma(0)
        x_dma(1)
        nc.sync.dma_start(wtT_sb[:], wtT_d.rearrange("p (k i) -> p k i", k=2))
        nc.sync.dma_start(wpT_sb[:], wpT_d.rearrange("p (k i) -> p k i", k=2))
        nc.sync.dma_start(wgT_sb[:], wgT_d.rearrange("p (k i) -> p k i", k=2))
        nc.sync.dma_start(wWT_sb[:], wWT_d)
        nc.sync.dma_start(bt_sb[:], bt_d)
        nc.sync.dma_start(bWp_sb[:], bWp_d)
        x_dma(2)

        # ---- PE warmup: keep the HAM activity window busy while the x DMA
        # streams in, so real matmuls start at 2.4 GHz instead of 1.2.
        warm_ps = ps.tile([P, NQ], F32, tag="s", name="warm_ps")
        for _ in range(WARMUP_MMS):
            nc.tensor.matmul(warm_ps[:, 0:P], ones_sb[:], ones_sb[:],
                             start=True, stop=True, skip_group_check=True)

        th_sb = big.tile([P, N], FP16, name="th_sb")   # theta^T (i, n)
        ph_sb = big.tile([P, N], FP16, name="ph_sb")   # phi (i, m)
        gT_sb = big.tile([P, N], BF16, name="gT_sb")   # g0^T (o, m)
        g_sb = big.tile([P, MC, P], BF16, name="g_sb")  # g0 (m_in, chunk, o)

        def proj_block(b, wT, dst, bias):
            sl = slice(b * 512, (b + 1) * 512)
            p_ps = ps.tile([P, NQ], F32, tag="s", name="p_ps")
            nc.tensor.matmul(p_ps[:, 0:512], wT[:, 0], x_sb[:, 0, sl],
                             start=True, stop=False)
            nc.tensor.matmul(p_ps[:, 0:512], wT[:, 1], x_sb[:, 1, sl],
                             start=False, stop=True)
            if bias is None:
                nc.scalar.copy(dst[:, sl], p_ps[:, 0:512])
            else:
                nc.scalar.activation(dst[:, sl], p_ps[:, 0:512], AF.Identity,
                                     bias=bias)

        # per-quarter state carried across the flat loop
        state = {}

        def start_quarter(q):
            state[q] = {
                "y": psy.tile([P, NQ], F32, tag="y", name=f"y{q}_ps"),
                "acc": [None, None],
                "exp": [None] * MC,
            }

        def emit_S(q, mc):
            st = state[q]
            msl = slice(mc * P, (mc + 1) * P)
            s_ps = ps.tile([P, NQ], F32, tag="s", name="s_ps")
            for b in range(NB):
                nc.tensor.matmul(
                    s_ps[:, b * 512:(b + 1) * 512], ph_sb[:, msl],
                    th_sb[:, q * NQ + b * 512: q * NQ + (b + 1) * 512],
                    start=True, stop=True)
            exp_t = work.tile([P, NQ], BF16, tag="exp", bufs=6, name="exp_sb")
            if mc in DVE_EXP_MCS:
                nc.vector.tensor_scalar(
                    exp_t[:].bitcast(U16), s_ps[:], SCHR_MUL, SCHR_ADD,
                    ALU.mult, ALU.add)
            else:
                nc.scalar.activation(exp_t[:], s_ps[:], AF.Exp,
                                     bias=cshift_sb[:, 0:1])
            st["exp"][mc] = exp_t

        def emit_AV(q, mc):
            st = state[q]
            exp_t = st["exp"][mc]
            for b in range(NB):
                bsl = slice(b * 512, (b + 1) * 512)
                nc.tensor.matmul(
                    st["y"][:, bsl], g_sb[:, mc], exp_t[:, bsl],
                    start=(mc == 0), stop=(mc == MC - 1),
                    skip_group_check=True)
            j = mc % 2
            if st["acc"][j] is None:
                st["acc"][j] = work.tile([P, NQ], BF16, tag=f"acc{j}",
                                         bufs=2, name=f"acc{j}_sb")
                nc.vector.tensor_copy(st["acc"][j][:], exp_t[:])
            else:
                nc.vector.tensor_add(st["acc"][j][:], st["acc"][j][:],
                                     exp_t[:])
            st["exp"][mc] = None

        def finish_quarter(q):
            # flush the 2-chunk AV skew, then fold the denominator partials
            emit_AV(q, MC - 2)
            emit_AV(q, MC - 1)
            st = state[q]
            sumt = ps.tile([P, NQ], F32, tag="s", name="sumt_ps")
            for b in range(NB):
                bsl = slice(b * 512, (b + 1) * 512)
                for j in range(2):
                    nc.tensor.matmul(sumt[:, bsl], ones_sb[:],
                                     st["acc"][j][:, bsl],
                                     start=(j == 0), stop=(j == 1),
                                     skip_group_check=True)
            st["sumt"] = sumt

        def emit_norm(q):
            """1/sums and normalized y^T, both 512-col blocks (DVE)."""
            st = state[q]
            st["recip"] = work.tile([P, NQ], F32, tag="recip", bufs=2,
                                    name="recip_sb")
            st["yt"] = work.tile([P, NQ], F32R, tag="yt", bufs=2,
                                 name="yt_sb")
            for b in range(NB):
                bsl = slice(b * 512, (b + 1) * 512)
                nc.vector.reciprocal_approx_fast(st["recip"][:, bsl],
                                                 st["sumt"][:, bsl])
                nc.vector.tensor_mul(st["yt"][:, bsl], st["y"][:, bsl],
                                     st["recip"][:, bsl])

        def emit_wy(q):
            st = state[q]
            st["wy"] = [ps.tile([P, NQ], F32, tag="s", name=f"wy{h}_ps")
                        for h in range(2)]
            for b in range(NB):
                bsl = slice(b * 512, (b + 1) * 512)
                for h in range(2):
                    nc.tensor.matmul(st["wy"][h][:, bsl],
                                     wWT_sb[:, h * P:(h + 1) * P],
                                     st["yt"][:, bsl], start=True, stop=True)

        def emit_o(q, add_engine):
            st = state[q]
            o = [work.tile([P, NQ], F32, tag=f"o{h}", bufs=2,
                           name=f"o{h}_sb") for h in range(2)]
            for b in range(NB):
                bsl = slice(b * 512, (b + 1) * 512)
                csl = slice(q * NQ + b * 512, q * NQ + (b + 1) * 512)
                for h in range(2):
                    nc.scalar.activation(o[h][:, bsl], st["wy"][h][:, bsl],
                                         AF.Identity, bias=bWp_sb[:, h:h + 1])
                    add_engine.tensor_add(o[h][:, bsl], o[h][:, bsl],
                                          x_sb[:, h, csl])
                    nc.sync.dma_start(out_v[h, :, csl], o[h][:, bsl])

        # ---- emission ----
        th_ready = 0

        def emit_th(b):
            proj_block(b, wtT_sb, th_sb, bt_sb[:, 0:1])

        emit_th(0)
        emit_th(1)

        for t in range(NQn * MC):
            q, mc = divmod(t, MC)
            if mc == 0:
                start_quarter(q)
            if q == 0 and mc % 4 == 0:
                b = mc // 4
                if b + 3 <= 7:
                    x_dma(b + 3)
                proj_block(b, wpT_sb, ph_sb, None)     # phi keys block
                if G_VIA_TRANSPOSE:
                    proj_block(b, wgT_sb, gT_sb, None)  # gT keys block
                    for m2 in range(4 * b, 4 * b + 4):  # g chunks via xbar
                        nc.sync.dma_start_transpose(
                            g_sb[:, m2], gT_sb[:, m2 * P:(m2 + 1) * P])
                else:
                    for m2 in range(4 * b, 4 * b + 4):
                        g_ps = ps.tile([P, NQ], F32, tag="s", name="g_ps")
                        msl = slice(m2 * P, (m2 + 1) * P)
                        nc.tensor.matmul(g_ps[:, 0:P], x_sb[:, 0, msl],
                                         wgT_sb[:, 0], start=True, stop=False)
                        nc.tensor.matmul(g_ps[:, 0:P], x_sb[:, 1, msl],
                                         wgT_sb[:, 1], start=False, stop=True)
                        if m2 % 2 == 0:
                            nc.vector.tensor_copy(g_sb[:, m2], g_ps[:, 0:P])
                        else:
                            nc.scalar.copy(g_sb[:, m2], g_ps[:, 0:P])
            if q < NQn - 1 and mc in (8, 16):
                emit_th(2 * (q + 1) + (mc == 16))
            # previous quarter's pipelined tail, emitted BEFORE this step's
            # psum allocations so pool recycling can't outrun its readers
            if q > 0:
                if mc == 2:
                    emit_norm(q - 1)
                elif mc == 3:
                    emit_wy(q - 1)
                elif mc == 4:
                    emit_o(q - 1, nc.vector)
                    del state[q - 1]
            emit_S(q, mc)
            if q > 0 and mc == 1:
                finish_quarter(q - 1)
            # AV skew: 2 behind normally; first AVs of q>0 wait for the
            # previous quarter's y_ps to be fully consumed (psy bufs=1)
            if q == 0:
                if mc >= 2:
                    emit_AV(q, mc - 2)
            else:
                if mc == 3:
                    emit_AV(q, 0)
                elif mc == 4:
                    emit_AV(q, 1)
                    emit_AV(q, 2)
                elif mc >= 5:
                    emit_AV(q, mc - 2)

        finish_quarter(NQn - 1)
        emit_norm(NQn - 1)
        emit_wy(NQn - 1)
        emit_o(NQn - 1, nc.vector)

    nc.compile()
    return nc


_CACHE = {}


def _built(key=(N_FULL,)):
    if key not in _CACHE:
        _CACHE[key] = build_nc(*key)
    return _CACHE[key]


def make_in_maps(x, wg, bg, wt, bt, wp, bp, wW, bW):
    """Host-side prep: per-core input dicts (core b <- batch b)."""
    x = np.asarray(x, np.float32)
    B, C_, H, W = x.shape
    N = H * W
    xf = np.ascontiguousarray(x.reshape(B, C_, N))
    wg, bg, wt, bt, wp, bp, wW, bW = [
        np.asarray(a, np.float32) for a in (wg, bg, wt, bt, wp, bp, wW, bW)]

    def pack(w):  # (128, C) conv weight -> partition-major lhsT chunks
        return np.ascontiguousarray(
            w.T.reshape(2, P, P).transpose(1, 0, 2).reshape(P, 2 * P))

    wtT, wpT, wgT = pack(wt), pack(wp), pack(wg)
    wWT = np.ascontiguousarray(wW.T)                       # (128, 256)
    bWp = (wW @ bg + bW).astype(np.float32)                # fold bg into bW
    bWp = np.ascontiguousarray(bWp.reshape(2, P).T)        # (128, 2)
    shared = {
        "wtT": wtT, "wpT": wpT, "wgT": wgT, "wWT": wWT,
        "bt": bt.reshape(P, 1).copy(), "bWp": bWp,
    }
    return [{"x": np.ascontiguousarray(xf[b]), **shared} for b in range(B)]


def kernel(x, wg, bg, wt, bt, wp, bp, wW, bW):
    from concourse.bass_utils import run_bass_kernel_spmd

    B, C_, H, W = np.asarray(x).shape
    in_maps = make_in_maps(x, wg, bg, wt, bt, wp, bp, wW, bW)
    nc = _built()
    res = run_bass_kernel_spmd(nc, in_maps, core_ids=list(range(B)))
    out = np.stack([res.results[b]["out"] for b in range(B)])
    return out.reshape(B, C_, H, W).astype(np.float32)


# revision 15
# speedup vs baseline: 1.1734x; 1.0297x over previous
"""NonLocalBlock (single-head attention, N=HW=4096, d=128) on 8 trn2 cores.

Sharding: data-parallel over batch (B=8) — one batch element per NeuronCore.
Per core, the whole block runs out of SBUF:

  xf (256, 4096) -> theta_T = wt@xf + bt      (128, N)   [PE + bias on copy]
                    phi     = wp@xf           (128, N)   [PE; bp dropped]
                    gT      = wg@xf           (128, N)   [PE]
                    g0      = gT^T chunks     (N, 128)   [xbar DMA transpose]
  S^T[m, n] = sum_i phi[i,m] * theta_T[i,n]   (keys m on partitions)
  expS = exp(S^T - 40)                         [ACT, some chunks DVE bit-trick]
  sums[n] = sum_m expS[m, n]                   [DVE bf16 partial adds + PE fold]
  yT[o, n] = (sum_m g0[m,o] expS[m,n]) / sums[n]
  out = wW @ yT + (wW@bg + bW) + xf

Numerics:
 - phi's bias bp only adds a per-query constant to S -> softmax-invariant,
   dropped entirely.
 - No per-row max: scores ~N(0,128), exp(S-40) stays in range (see analysis).
 - theta/phi stored fp16: stationary operand gets FWL (2x faster weight
   load); fp16 keeps the absolute score error ~3e-3 (negligible through exp).
 - DVE_EXP_MCS key-chunks per quarter compute exp on the Vector engine via
   the Schraudolph bit trick: bf16_bits(e^x) ~= rint(x*128*log2e + 16256) as
   one tensor_scalar (fp32 PSUM -> uint16, HW round-to-nearest + saturate),
   bitcast to bf16.  ~3.3% max rel err on those chunks' weights; offloads
   the otherwise-bottleneck ACT exp stream.

Schedule: flat 128-step loop (4 query-quarters x 32 key-chunks).  AV matmuls
run 2 chunks behind S for elasticity; each quarter's tail (fold / reciprocal
/ normalize / Wy / +x / store) is emitted inside the next quarter's first
steps in 512-column blocks so no engine drains; projections stream behind
the x DMA; dummy PE warmup keeps HAM from throttling the prologue.
"""

import numpy as np
from contextlib import ExitStack

import concourse.bass as bass
import concourse.mybir as mybir
import concourse.tile as tile
from concourse import bacc

P = 128          # partitions / inter channels
C = 256          # input channels
F32 = mybir.dt.float32
F32R = mybir.dt.float32r
FP16 = mybir.dt.float16
U16 = mybir.dt.uint16
BF16 = mybir.dt.bfloat16
AF = mybir.ActivationFunctionType
ALU = mybir.AluOpType
CSHIFT = 40.0    # global score shift before exp (softmax-invariant)

LOG2E = 1.4426950408889634
SCHR_MUL = float(np.float32(128 * LOG2E))
SCHR_ADD = float(np.float32(16256 - CSHIFT * 128 * LOG2E - 5.61))

B_FULL = 8
H_FULL = 64
W_FULL = 64
N_FULL = H_FULL * W_FULL

NQ = 1024                     # query-quarter width
# key-chunks (of 32 per quarter) whose exp runs on DVE instead of ACT
DVE_EXP_MCS = (4, 11, 17, 23, 27)
WARMUP_MMS = 20               # keep PE ticking until the first x block lands
G_VIA_TRANSPOSE = False       # gT + xbar transpose vs per-chunk matmuls


def build_nc(N=N_FULL):
    MC = N // P                   # 32 key chunks
    NQn = N // NQ                 # 4 query quarters
    NB = NQ // 512                # 2 512-wide blocks per quarter

    nc = bacc.Bacc("TRN2", target_bir_lowering=False, debug=False)

    x_d = nc.dram_tensor("x", [C, N], F32R, kind="ExternalInput").ap()
    wtT_d = nc.dram_tensor("wtT", [P, 2 * P], F32R, kind="ExternalInput").ap()
    wpT_d = nc.dram_tensor("wpT", [P, 2 * P], F32R, kind="ExternalInput").ap()
    wgT_d = nc.dram_tensor("wgT", [P, 2 * P], F32R, kind="ExternalInput").ap()
    wWT_d = nc.dram_tensor("wWT", [P, C], F32R, kind="ExternalInput").ap()
    bt_d = nc.dram_tensor("bt", [P, 1], F32, kind="ExternalInput").ap()
    bWp_d = nc.dram_tensor("bWp", [P, 2], F32, kind="ExternalInput").ap()
    out_d = nc.dram_tensor("out", [C, N], F32, kind="ExternalOutput").ap()

    x_v = x_d.rearrange("(k p) n -> k p n", p=P)
    out_v = out_d.rearrange("(k p) n -> k p n", p=P)

    with tile.TileContext(nc) as tc, ExitStack() as ctx:
        const = ctx.enter_context(tc.tile_pool(name="const", bufs=1))
        big = ctx.enter_context(tc.tile_pool(name="big", bufs=1))
        work = ctx.enter_context(tc.tile_pool(name="work", bufs=3))
        ps = ctx.enter_context(tc.tile_pool(name="ps", bufs=3, space="PSUM"))
        psy = ctx.enter_context(tc.tile_pool(name="psy", bufs=1, space="PSUM"))

        # ---- constants ----
        wtT_sb = const.tile([P, 2, P], F32R, name="wtT_sb")
        wpT_sb = const.tile([P, 2, P], F32R, name="wpT_sb")
        wgT_sb = const.tile([P, 2, P], F32R, name="wgT_sb")
        wWT_sb = const.tile([P, C], F32R, name="wWT_sb")
        bt_sb = const.tile([P, 1], F32, name="bt_sb")
        bWp_sb = const.tile([P, 2], F32, name="bWp_sb")
        ones_sb = const.tile([P, P], BF16, name="ones_sb")
        cshift_sb = const.tile([P, 1], F32, name="cshift_sb")
        nc.vector.memset(cshift_sb[:], -CSHIFT)
        nc.vector.memset(ones_sb[:], 1.0)

        x_sb = big.tile([P, 2, N], F32R, name="x_sb")

        def x_dma(b):
            for k in range(2):
                nc.sync.dma_start(
                    x_sb[:, k, b * 512:(b + 1) * 512],
                    x_v[k, :, b * 512:(b + 1) * 512],
                )

        # x triggers on the SP queue; weights go out on the Activation
        # engine's HWDGE queue in parallel (each dma_start trigger costs
        # ~0.6us of queue time, so serializing all of them delays theta)
        x_dma(0)
        x_dma(1)
        nc.scalar.dma_start(wtT_sb[:], wtT_d.rearrange("p (k i) -> p k i", k=2))
        nc.scalar.dma_start(wpT_sb[:], wpT_d.rearrange("p (k i) -> p k i", k=2))
        nc.scalar.dma_start(wgT_sb[:], wgT_d.rearrange("p (k i) -> p k i", k=2))
        nc.scalar.dma_start(wWT_sb[:], wWT_d)
        nc.scalar.dma_start(bt_sb[:], bt_d)
        nc.scalar.dma_start(bWp_sb[:], bWp_d)
        x_dma(2)

        # ---- PE warmup: keep the HAM activity window busy while the x DMA
        # streams in, so real matmuls start at 2.4 GHz instead of 1.2.
        warm_ps = ps.tile([P, NQ], F32, tag="s", name="warm_ps")
        for _ in range(WARMUP_MMS):
            nc.tensor.matmul(warm_ps[:, 0:P], ones_sb[:], ones_sb[:],
                             start=True, stop=True, skip_group_check=True)

        th_sb = big.tile([P, N], FP16, name="th_sb")   # theta^T (i, n)
        ph_sb = big.tile([P, N], FP16, name="ph_sb")   # phi (i, m)
        gT_sb = big.tile([P, N], BF16, name="gT_sb")   # g0^T (o, m)
        g_sb = big.tile([P, MC, P], BF16, name="g_sb")  # g0 (m_in, chunk, o)

        def proj_block(b, wT, dst, bias):
            sl = slice(b * 512, (b + 1) * 512)
            p_ps = ps.tile([P, NQ], F32, tag="s", name="p_ps")
            nc.tensor.matmul(p_ps[:, 0:512], wT[:, 0], x_sb[:, 0, sl],
                             start=True, stop=False)
            nc.tensor.matmul(p_ps[:, 0:512], wT[:, 1], x_sb[:, 1, sl],
                             start=False, stop=True)
            if bias is None:
                nc.scalar.copy(dst[:, sl], p_ps[:, 0:512])
            else:
                nc.scalar.activation(dst[:, sl], p_ps[:, 0:512], AF.Identity,
                                     bias=bias)

        # per-quarter state carried across the flat loop
        state = {}

        def start_quarter(q):
            state[q] = {
                "y": psy.tile([P, NQ], F32, tag="y", name=f"y{q}_ps"),
                "acc": [None, None],
                "exp": [None] * MC,
            }

        def emit_S(q, mc):
            st = state[q]
            msl = slice(mc * P, (mc + 1) * P)
            s_ps = ps.tile([P, NQ], F32, tag="s", name="s_ps")
            for b in range(NB):
                nc.tensor.matmul(
                    s_ps[:, b * 512:(b + 1) * 512], ph_sb[:, msl],
                    th_sb[:, q * NQ + b * 512: q * NQ + (b + 1) * 512],
                    start=True, stop=True)
            exp_t = work.tile([P, NQ], BF16, tag="exp", bufs=7, name="exp_sb")
            if mc in DVE_EXP_MCS:
                nc.vector.tensor_scalar(
                    exp_t[:].bitcast(U16), s_ps[:], SCHR_MUL, SCHR_ADD,
                    ALU.mult, ALU.add)
            else:
                nc.scalar.activation(exp_t[:], s_ps[:], AF.Exp,
                                     bias=cshift_sb[:, 0:1])
            st["exp"][mc] = exp_t

        def emit_AV(q, mc):
            st = state[q]
            exp_t = st["exp"][mc]
            for b in range(NB):
                bsl = slice(b * 512, (b + 1) * 512)
                nc.tensor.matmul(
                    st["y"][:, bsl], g_sb[:, mc], exp_t[:, bsl],
                    start=(mc == 0), stop=(mc == MC - 1),
                    skip_group_check=True)
            j = mc % 2
            if st["acc"][j] is None:
                st["acc"][j] = work.tile([P, NQ], BF16, tag=f"acc{j}",
                                         bufs=2, name=f"acc{j}_sb")
                nc.vector.tensor_copy(st["acc"][j][:], exp_t[:])
            else:
                nc.vector.tensor_add(st["acc"][j][:], st["acc"][j][:],
                                     exp_t[:])
            st["exp"][mc] = None

        def finish_quarter(q):
            # flush the AV skew, then fold the denominator partials
            emit_AV(q, MC - 3)
            emit_AV(q, MC - 2)
            emit_AV(q, MC - 1)
            st = state[q]
            sumt = ps.tile([P, NQ], F32, tag="s", name="sumt_ps")
            for b in range(NB):
                bsl = slice(b * 512, (b + 1) * 512)
                for j in range(2):
                    nc.tensor.matmul(sumt[:, bsl], ones_sb[:],
                                     st["acc"][j][:, bsl],
                                     start=(j == 0), stop=(j == 1),
                                     skip_group_check=True)
            st["sumt"] = sumt

        def emit_norm(q):
            """1/sums and normalized y^T, both 512-col blocks (DVE)."""
            st = state[q]
            st["recip"] = work.tile([P, NQ], F32, tag="recip", bufs=2,
                                    name="recip_sb")
            st["yt"] = work.tile([P, NQ], F32R, tag="yt", bufs=2,
                                 name="yt_sb")
            for b in range(NB):
                bsl = slice(b * 512, (b + 1) * 512)
                nc.vector.reciprocal_approx_fast(st["recip"][:, bsl],
                                                 st["sumt"][:, bsl])
                nc.vector.tensor_mul(st["yt"][:, bsl], st["y"][:, bsl],
                                     st["recip"][:, bsl])

        def emit_wy(q):
            st = state[q]
            st["wy"] = [ps.tile([P, NQ], F32, tag="s", name=f"wy{h}_ps")
                        for h in range(2)]
            for b in range(NB):
                bsl = slice(b * 512, (b + 1) * 512)
                for h in range(2):
                    nc.tensor.matmul(st["wy"][h][:, bsl],
                                     wWT_sb[:, h * P:(h + 1) * P],
                                     st["yt"][:, bsl], start=True, stop=True)

        def emit_o(q, add_engine):
            st = state[q]
            o = [work.tile([P, NQ], F32, tag=f"o{h}", bufs=2,
                           name=f"o{h}_sb") for h in range(2)]
            for b in range(NB):
                bsl = slice(b * 512, (b + 1) * 512)
                csl = slice(q * NQ + b * 512, q * NQ + (b + 1) * 512)
                for h in range(2):
                    nc.scalar.activation(o[h][:, bsl], st["wy"][h][:, bsl],
                                         AF.Identity, bias=bWp_sb[:, h:h + 1])
                    add_engine.tensor_add(o[h][:, bsl], o[h][:, bsl],
                                          x_sb[:, h, csl])
                    nc.sync.dma_start(out_v[h, :, csl], o[h][:, bsl])

        # ---- emission ----
        th_ready = 0

        def emit_th(b):
            proj_block(b, wtT_sb, th_sb, bt_sb[:, 0:1])

        emit_th(0)
        emit_th(1)

        for t in range(NQn * MC):
            q, mc = divmod(t, MC)
            if mc == 0:
                start_quarter(q)
            if q == 0 and mc % 4 == 0:
                b = mc // 4
                if b + 3 <= 7:
                    x_dma(b + 3)
                proj_block(b, wpT_sb, ph_sb, None)     # phi keys block
                if G_VIA_TRANSPOSE:
                    proj_block(b, wgT_sb, gT_sb, None)  # gT keys block
                    for m2 in range(4 * b, 4 * b + 4):  # g chunks via xbar
                        nc.sync.dma_start_transpose(
                            g_sb[:, m2], gT_sb[:, m2 * P:(m2 + 1) * P])
                else:
                    for m2 in range(4 * b, 4 * b + 4):
                        g_ps = ps.tile([P, NQ], F32, tag="s", name="g_ps")
                        msl = slice(m2 * P, (m2 + 1) * P)
                        nc.tensor.matmul(g_ps[:, 0:P], x_sb[:, 0, msl],
                                         wgT_sb[:, 0], start=True, stop=False)
                        nc.tensor.matmul(g_ps[:, 0:P], x_sb[:, 1, msl],
                                         wgT_sb[:, 1], start=False, stop=True)
                        if m2 % 2 == 0:
                            nc.vector.tensor_copy(g_sb[:, m2], g_ps[:, 0:P])
                        else:
                            nc.scalar.copy(g_sb[:, m2], g_ps[:, 0:P])
            if q < NQn - 1 and mc in (8, 16):
                emit_th(2 * (q + 1) + (mc == 16))
            # previous quarter's pipelined tail, emitted BEFORE this step's
            # psum allocations so pool recycling can't outrun its readers
            if q > 0:
                if mc == 2:
                    emit_norm(q - 1)
                elif mc == 3:
                    emit_wy(q - 1)
                elif mc == 4:
                    emit_o(q - 1, nc.vector)
                    del state[q - 1]
            emit_S(q, mc)
            if q > 0 and mc == 1:
                finish_quarter(q - 1)
            # AV skew: 3 behind normally; first AVs of q>0 wait for the
            # previous quarter's y_ps to be fully consumed (psy bufs=1)
            if q == 0:
                if mc >= 3:
                    emit_AV(q, mc - 3)
            else:
                if mc in (3, 4, 5):
                    emit_AV(q, mc - 3)
                elif mc >= 6:
                    emit_AV(q, mc - 3)

        # final quarter: block-pipelined tail so the first 512 columns hit
        # the output DMA while the second block is still normalizing
        qf = NQn - 1
        finish_quarter(qf)
        st = state[qf]
        st["recip"] = work.tile([P, NQ], F32, tag="recip", bufs=2,
                                name="recip_sb")
        st["yt"] = work.tile([P, NQ], F32R, tag="yt", bufs=2, name="yt_sb")
        st["wy"] = [ps.tile([P, NQ], F32, tag="s", name=f"wyf{h}_ps")
                    for h in range(2)]
        of = [work.tile([P, NQ], F32, tag=f"o{h}", bufs=2, name=f"of{h}_sb")
              for h in range(2)]
        for b in range(NB):
            bsl = slice(b * 512, (b + 1) * 512)
            csl = slice(qf * NQ + b * 512, qf * NQ + (b + 1) * 512)
            nc.vector.reciprocal_approx_fast(st["recip"][:, bsl],
                                             st["sumt"][:, bsl])
            nc.vector.tensor_mul(st["yt"][:, bsl], st["y"][:, bsl],
                                 st["recip"][:, bsl])
            for h in range(2):
                nc.tensor.matmul(st["wy"][h][:, bsl],
                                 wWT_sb[:, h * P:(h + 1) * P],
                                 st["yt"][:, bsl], start=True, stop=True)
            for h in range(2):
                nc.scalar.activation(of[h][:, bsl], st["wy"][h][:, bsl],
                                     AF.Identity, bias=bWp_sb[:, h:h + 1])
                nc.vector.tensor_add(of[h][:, bsl], of[h][:, bsl],
                                     x_sb[:, h, csl])
                nc.sync.dma_start(out_v[h, :, csl], of[h][:, bsl])

    nc.compile()
    return nc


_CACHE = {}


def _built(key=(N_FULL,)):
    if key not in _CACHE:
        _CACHE[key] = build_nc(*key)
    return _CACHE[key]


def make_in_maps(x, wg, bg, wt, bt, wp, bp, wW, bW):
    """Host-side prep: per-core input dicts (core b <- batch b)."""
    x = np.asarray(x, np.float32)
    B, C_, H, W = x.shape
    N = H * W
    xf = np.ascontiguousarray(x.reshape(B, C_, N))
    wg, bg, wt, bt, wp, bp, wW, bW = [
        np.asarray(a, np.float32) for a in (wg, bg, wt, bt, wp, bp, wW, bW)]

    def pack(w):  # (128, C) conv weight -> partition-major lhsT chunks
        return np.ascontiguousarray(
            w.T.reshape(2, P, P).transpose(1, 0, 2).reshape(P, 2 * P))

    wtT, wpT, wgT = pack(wt), pack(wp), pack(wg)
    wWT = np.ascontiguousarray(wW.T)                       # (128, 256)
    bWp = (wW @ bg + bW).astype(np.float32)                # fold bg into bW
    bWp = np.ascontiguousarray(bWp.reshape(2, P).T)        # (128, 2)
    shared = {
        "wtT": wtT, "wpT": wpT, "wgT": wgT, "wWT": wWT,
        "bt": bt.reshape(P, 1).copy(), "bWp": bWp,
    }
    return [{"x": np.ascontiguousarray(xf[b]), **shared} for b in range(B)]


def kernel(x, wg, bg, wt, bt, wp, bp, wW, bW):
    from concourse.bass_utils import run_bass_kernel_spmd

    B, C_, H, W = np.asarray(x).shape
    in_maps = make_in_maps(x, wg, bg, wt, bt, wp, bp, wW, bW)
    nc = _built()
    res = run_bass_kernel_spmd(nc, in_maps, core_ids=list(range(B)))
    out = np.stack([res.results[b]["out"] for b in range(B)])
    return out.reshape(B, C_, H, W).astype(np.float32)
